# revision 10
# baseline (speedup 1.0000x reference)
"""DLinear forward, folded to a single mat-vec, on 8 TRN2 NeuronCores.

The reference network is linear in x:
    out[b] = sum_{l,c} x[b,l,c] * W[c,l] + const
where W folds the moving-average (edge-padded, window 25), both per-channel
linears and the decoder. W/const are computed on host in float64 (tiny,
weights-only); the 662MB x never leaves the device path: each core streams
its batch shard and computes a fused multiply+reduce (DVE tensor_tensor_reduce)
against the folded vector v broadcast across partitions via a PE ones-matmul.
"""

import sys

import numpy as np

for _p in ("/opt/trn_rl_repo",):
    if _p not in sys.path:
        sys.path.insert(0, _p)

_B, _L, _C = 2048, 512, 158
_K = 25
_PAD = (_K - 1) // 2
_NCORES = 8
_BS = _B // _NCORES           # 256 rows per core
_F = _L * _C                  # 80896 features
_FC = 2048                    # features per chunk
_NCHUNKS = (_F + _FC - 1) // _FC   # 40 (39 full + 1 of 1024)
_CPQ = 14                     # chunks per quadrant row (3 rows: 14+14+12)
_ONES_OFF = _CPQ * _FC        # ones block starts after the chunks in a row
_VROW = _ONES_OFF + 128       # row width of the staged v layout


def _fold_weights(w_seasonal, b_seasonal, w_trend, b_trend, w_dec, b_dec):
    w_s = np.asarray(w_seasonal, np.float64)
    w_t = np.asarray(w_trend, np.float64)
    b_s = np.asarray(b_seasonal, np.float64)
    b_t = np.asarray(b_trend, np.float64)
    w_d = np.asarray(w_dec, np.float64)
    b_d = float(np.asarray(b_dec, np.float64))
    C, L = w_s.shape
    # M[l, lp] = #{d in [-p, p] : clamp(l+d, 0, L-1) == lp}: the linear map of
    # the edge-padded moving average, so that sum_l trend[.,l]*g[l] ==
    # sum_lp x[.,lp] * (g @ M)[lp] / K exactly.
    M = np.zeros((L, L))
    for l in range(L):
        for d in range(-_PAD, _PAD + 1):
            M[l, min(max(l + d, 0), L - 1)] += 1.0
    Wcomb = w_s + ((w_t - w_s) @ M) / _K        # [C, L]
    W = Wcomb * w_d[:, None]                    # [C, L]
    v = np.ascontiguousarray(W.T).reshape(-1).astype(np.float32)  # index l*C+c
    const = float(np.sum(w_d * (b_s + b_t)) + b_d)
    return v, const


def _build(const):
    from contextlib import ExitStack

    import concourse.bacc as bacc
    import concourse.mybir as mybir
    import concourse.tile as tile

    f32 = mybir.dt.float32
    nc = bacc.Bacc(None, target_bir_lowering=False)
    x = nc.dram_tensor("x", [_BS, _F], f32, kind="ExternalInput")
    # vb row q (for SBUF partition 32q) = 14 v-chunks + a 128-wide ones block.
    # PE matmul needs lhsT/rhs on the same base partition in {0, 32, 64}.
    vb = nc.dram_tensor("vb", [3, _VROW], f32, kind="ExternalInput")
    y = nc.dram_tensor("y", [_BS, 1], f32, kind="ExternalOutput")

    # partitions p = batch row within a half; g = which half of the shard
    xv = x[:, :].rearrange("(g p) f -> p g f", p=128)

    with tile.TileContext(nc) as tc, ExitStack() as ctx:
        xpool = ctx.enter_context(tc.tile_pool(name="xp", bufs=4))
        ppool = ctx.enter_context(tc.tile_pool(name="pp", bufs=2, space="PSUM"))
        spool = ctx.enter_context(tc.tile_pool(name="sp", bufs=1))

        vs = spool.tile([65, _VROW], f32)
        nc.scalar.dma_start(out=vs[::32, :], in_=vb[:, :])
        acc = spool.tile([128, 2 * _NCHUNKS], f32)
        dummy = spool.tile([128, 1], f32)
        res = spool.tile([128, 2], f32)

        for i in range(_NCHUNKS):
            f0 = i * _FC
            fc = min(_FC, _F - f0)
            q = 32 * (i // _CPQ)
            col = (i % _CPQ) * _FC
            ones = vs[q:q + 1, _ONES_OFF:_ONES_OFF + 128]
            # replicate v-chunk to all 128 partitions: psum = ones.T @ v_chunk
            pv = ppool.tile([128, _FC], f32)
            for j in range(0, fc, 512):
                w = min(512, fc - j)
                nc.tensor.matmul(pv[:, j:j + w], ones,
                                 vs[q:q + 1, col + j:col + j + w],
                                 start=True, stop=True)
            xt = xpool.tile([128, 2, _FC], f32)
            nc.sync.dma_start(out=xt[:, :, :fc], in_=xv[:, :, f0:f0 + fc])
            for g in range(2):
                # acc[:, col] = sum_f(x * v_chunk), fused multiply+reduce
                nc.vector.scalar_tensor_tensor(
                    out=dummy.broadcast_to((128, fc)),
                    in0=xt[:, g, :fc],
                    scalar=1.0,
                    in1=pv[:, :fc],
                    op0=mybir.AluOpType.mult,
                    op1=mybir.AluOpType.mult,
                    accum_out=acc[:, g * _NCHUNKS + i: g * _NCHUNKS + i + 1],
                )
        for g in range(2):
            nc.vector.tensor_reduce(
                out=res[:, g:g + 1],
                in_=acc[:, g * _NCHUNKS:(g + 1) * _NCHUNKS],
                axis=mybir.AxisListType.X,
                op=mybir.AluOpType.add,
            )
            nc.vector.tensor_scalar_add(res[:, g:g + 1], res[:, g:g + 1], const)
            nc.sync.dma_start(out=y[g * 128:(g + 1) * 128, :],
                              in_=res[:, g:g + 1])
    nc.compile()
    return nc


def kernel(**inputs):
    x = np.ascontiguousarray(np.asarray(inputs["x"], dtype=np.float32))
    assert x.shape == (_B, _L, _C), x.shape
    v, const = _fold_weights(
        inputs["w_seasonal"], inputs["b_seasonal"],
        inputs["w_trend"], inputs["b_trend"],
        inputs["w_dec"], inputs["b_dec"],
    )
    nc = _build(const)

    from concourse.bass_utils import run_bass_kernel_spmd

    vpad = np.zeros(_NCHUNKS * _FC, np.float32)
    vpad[:_F] = v
    vb = np.zeros((3, _VROW), np.float32)
    for i in range(_NCHUNKS):
        q, k = divmod(i, _CPQ)
        vb[q, k * _FC:(k + 1) * _FC] = vpad[i * _FC:(i + 1) * _FC]
    vb[:, _ONES_OFF:_ONES_OFF + 128] = 1.0
    x2 = x.reshape(_B, _F)
    in_maps = [
        {"x": np.ascontiguousarray(x2[i * _BS:(i + 1) * _BS]), "vb": vb}
        for i in range(_NCORES)
    ]
    r = run_bass_kernel_spmd(nc, in_maps, core_ids=list(range(_NCORES)))
    kernel._last = r
    out = np.concatenate([r.results[i]["y"].reshape(-1) for i in range(_NCORES)])
    return out.astype(np.float32, copy=False)


# revision 14
# speedup vs baseline: 1.1193x; 1.1193x over previous
"""DLinear forward, folded to a single mat-vec, on 8 TRN2 NeuronCores.

The reference network is linear in x:
    out[b] = sum_{l,c} x[b,l,c] * W[c,l] + const
where W folds the moving-average (edge-padded, window 25), both per-channel
linears and the decoder. W/const are computed on host in float64 (tiny,
weights-only); the 662MB x never leaves the device path: each core streams
its batch shard and computes a fused multiply+reduce (DVE tensor_tensor_reduce)
against the folded vector v broadcast across partitions via a PE ones-matmul.
"""

import sys

import numpy as np

for _p in ("/opt/trn_rl_repo",):
    if _p not in sys.path:
        sys.path.insert(0, _p)

_B, _L, _C = 2048, 512, 158
_K = 25
_PAD = (_K - 1) // 2
_NCORES = 8
_BS = _B // _NCORES           # 256 rows per core
_F = _L * _C                  # 80896 features
_FC = 2048                    # features per chunk
_NCHUNKS = (_F + _FC - 1) // _FC   # 40 (39 full + 1 of 1024)
_CPQ = 14                     # chunks per quadrant row (3 rows: 14+14+12)
_ONES_OFF = _CPQ * _FC        # ones block starts after the chunks in a row
_VROW = _ONES_OFF + 128       # row width of the staged v layout


def _fold_weights(w_seasonal, b_seasonal, w_trend, b_trend, w_dec, b_dec):
    w_s = np.asarray(w_seasonal, np.float64)
    w_t = np.asarray(w_trend, np.float64)
    b_s = np.asarray(b_seasonal, np.float64)
    b_t = np.asarray(b_trend, np.float64)
    w_d = np.asarray(w_dec, np.float64)
    b_d = float(np.asarray(b_dec, np.float64))
    C, L = w_s.shape
    # M[l, lp] = #{d in [-p, p] : clamp(l+d, 0, L-1) == lp}: the linear map of
    # the edge-padded moving average, so that sum_l trend[.,l]*g[l] ==
    # sum_lp x[.,lp] * (g @ M)[lp] / K exactly.
    M = np.zeros((L, L))
    for l in range(L):
        for d in range(-_PAD, _PAD + 1):
            M[l, min(max(l + d, 0), L - 1)] += 1.0
    Wcomb = w_s + ((w_t - w_s) @ M) / _K        # [C, L]
    W = Wcomb * w_d[:, None]                    # [C, L]
    v = np.ascontiguousarray(W.T).reshape(-1).astype(np.float32)  # index l*C+c
    const = float(np.sum(w_d * (b_s + b_t)) + b_d)
    return v, const


def _build(const):
    from contextlib import ExitStack

    import concourse.bacc as bacc
    import concourse.mybir as mybir
    import concourse.tile as tile

    f32 = mybir.dt.float32
    bf16 = mybir.dt.bfloat16
    nc = bacc.Bacc(None, target_bir_lowering=False)
    x = nc.dram_tensor("x", [_BS, _F], f32, kind="ExternalInput")
    # vb row q (for SBUF partition 32q) = 14 v-chunks + a 128-wide ones block,
    # as a bf16 hi/lo pair (h=0 hi, h=1 lo): v = hi + lo reconstructed by two
    # PE matmuls accumulating into the same PSUM bank (bf16 runs 4x faster
    # than fp32 on the PE and the f32 PSUM sum is near-exact).
    # PE matmul needs lhsT/rhs on the same base partition in {0, 32, 64}.
    vb = nc.dram_tensor("vb", [3, 2, _VROW], bf16, kind="ExternalInput")
    y = nc.dram_tensor("y", [_BS, 1], f32, kind="ExternalOutput")

    # partitions p = batch row within a half; g = which half of the shard
    xv = x[:, :].rearrange("(g p) f -> p g f", p=128)

    with tile.TileContext(nc) as tc, ExitStack() as ctx:
        xpool = ctx.enter_context(tc.tile_pool(name="xp", bufs=4))
        ppool = ctx.enter_context(tc.tile_pool(name="pp", bufs=2, space="PSUM"))
        spool = ctx.enter_context(tc.tile_pool(name="sp", bufs=1))

        vs = spool.tile([65, 2, _VROW], bf16)
        nc.scalar.dma_start(out=vs[::32, :, :], in_=vb[:, :, :])
        acc = spool.tile([128, 2 * _NCHUNKS], f32)
        dummy = spool.tile([128, 1], f32)
        res = spool.tile([128, 2], f32)

        for i in range(_NCHUNKS):
            f0 = i * _FC
            fc = min(_FC, _F - f0)
            q = 32 * (i // _CPQ)
            col = (i % _CPQ) * _FC
            ones = vs[q:q + 1, 0, _ONES_OFF:_ONES_OFF + 128]
            # replicate v-chunk to all 128 partitions: psum = ones.T @ v_chunk
            pv = ppool.tile([128, _FC], f32)
            for j in range(0, fc, 512):
                w = min(512, fc - j)
                nc.tensor.matmul(pv[:, j:j + w], ones,
                                 vs[q:q + 1, 0, col + j:col + j + w],
                                 start=True, stop=False)
                nc.tensor.matmul(pv[:, j:j + w], ones,
                                 vs[q:q + 1, 1, col + j:col + j + w],
                                 start=False, stop=True)
            xt = xpool.tile([128, 2, _FC], f32)
            nc.sync.dma_start(out=xt[:, :, :fc], in_=xv[:, :, f0:f0 + fc])
            for g in range(2):
                # acc[:, col] = sum_f(x * v_chunk), fused multiply+reduce
                nc.vector.scalar_tensor_tensor(
                    out=dummy.broadcast_to((128, fc)),
                    in0=xt[:, g, :fc],
                    scalar=1.0,
                    in1=pv[:, :fc],
                    op0=mybir.AluOpType.mult,
                    op1=mybir.AluOpType.mult,
                    accum_out=acc[:, g * _NCHUNKS + i: g * _NCHUNKS + i + 1],
                )
        for g in range(2):
            nc.vector.tensor_reduce(
                out=res[:, g:g + 1],
                in_=acc[:, g * _NCHUNKS:(g + 1) * _NCHUNKS],
                axis=mybir.AxisListType.X,
                op=mybir.AluOpType.add,
            )
            nc.vector.tensor_scalar_add(res[:, g:g + 1], res[:, g:g + 1], const)
            nc.sync.dma_start(out=y[g * 128:(g + 1) * 128, :],
                              in_=res[:, g:g + 1])
    nc.compile()
    return nc


def _pack_vb(v):
    import ml_dtypes

    vpad = np.zeros(_NCHUNKS * _FC, np.float32)
    vpad[:len(v)] = v
    vrows = np.zeros((3, _VROW), np.float32)
    for i in range(_NCHUNKS):
        q, k = divmod(i, _CPQ)
        vrows[q, k * _FC:(k + 1) * _FC] = vpad[i * _FC:(i + 1) * _FC]
    v_hi = vrows.astype(ml_dtypes.bfloat16)
    v_lo = (vrows - v_hi.astype(np.float32)).astype(ml_dtypes.bfloat16)
    vb = np.zeros((3, 2, _VROW), ml_dtypes.bfloat16)
    vb[:, 0, :] = v_hi
    vb[:, 1, :] = v_lo
    vb[:, 0, _ONES_OFF:_ONES_OFF + 128] = 1.0
    vb[:, 1, _ONES_OFF:_ONES_OFF + 128] = 0.0
    return vb


def kernel(**inputs):
    x = np.ascontiguousarray(np.asarray(inputs["x"], dtype=np.float32))
    assert x.shape == (_B, _L, _C), x.shape
    v, const = _fold_weights(
        inputs["w_seasonal"], inputs["b_seasonal"],
        inputs["w_trend"], inputs["b_trend"],
        inputs["w_dec"], inputs["b_dec"],
    )
    nc = _build(const)

    from concourse.bass_utils import run_bass_kernel_spmd

    vb = _pack_vb(v)
    x2 = x.reshape(_B, _F)
    in_maps = [
        {"x": np.ascontiguousarray(x2[i * _BS:(i + 1) * _BS]), "vb": vb}
        for i in range(_NCORES)
    ]
    r = run_bass_kernel_spmd(nc, in_maps, core_ids=list(range(_NCORES)))
    kernel._last = r
    out = np.concatenate([r.results[i]["y"].reshape(-1) for i in range(_NCORES)])
    return out.astype(np.float32, copy=False)
